# revision 9
# baseline (speedup 1.0000x reference)
"""Multi-head causal attention (B=2, S=4096, D=768, H=12) on 8 Trainium2 cores.

Sharding: one (batch, 3-head group) pair per core — batch b = core//4,
heads 3*(core%4) .. +2.  Wq/Wk/Wv split column-wise (per head group),
Wo row-wise; each core emits a partial [S, D] output which the host sums
per batch and adds bo.

Device kernel (per core), per 512-query block j:
  1. stream qT/kT/vT block [768, 512] bf16 (host pre-transposes)
  2. Q/K projections -> transposed-head layout: qh01/kh01 [128, 512]
     (heads 0,1 stacked on partitions) and qh2d/kh2d [128, 512] (head 2
     DUPLICATED on both partition halves via duplicated weight columns).
     This lets every score matmul pair (rows 0-63 vs rows 64-127) run
     CONCURRENTLY in the PE array via row-group tiling.
  3. Vh' [s, 3, 65] natural layout (ones column -> softmax denominators
     accumulate for free in the PV matmul).
  4. attention tiles processed in 3-tile PSUM slots [128, 3, 512]
     (3 banks): scores (K=64, row-tiled pairs), exp, PV (K=128).
     exp runs on EITHER ScalarE (table exp) or VectorE (custom EXP4A op:
     (p3(x))^4 ~ exp(x/8), 8 ALU stages) -- greedy load balancing between
     the two engines.  Causal mask on diagonal tiles via precomputed 0/1
     bf16 masks (DVE mul); diagonal exps are column-restricted.
  5. normalize: reciprocal of denom row (DVE), partition_broadcast
     (GpSimd), multiply (DVE) -> feats bf16
  6. Wo: out[sq,768] partial via K=64 bf16 matmuls; PSUM->SBUF copy on
     ACT or DVE (balanced), DMA out.
"""

import numpy as np
import ml_dtypes

import concourse.bass as bass
import concourse.mybir as mybir
from concourse.tile import TileContext
from bass_rust import ScopedClock

B, S, D, H = 2, 4096, 768, 12
HD = D // H  # 64
N_CORES = 8
CORES_PER_BATCH = 4
HPC = H // CORES_PER_BATCH  # heads per core = 3
HB = HPC * HD  # head-block width = 192
SQ = 512  # query-column block (matmul moving free dim)
F32 = mybir.dt.float32
BF16 = mybir.dt.bfloat16
AF = mybir.ActivationFunctionType

# (p3(x))^4 ~ exp(x/8) for x in [-26, 26]; p3 ~ exp(x/32), rel err < 1.3%
EXP4_C = (9.98648338e-01, 3.13934606e-02, 5.11392455e-04, 4.88074139e-06)


def _register_exp4():
    """Register the EXP4A custom DVE op (idempotent, runtime registration)."""
    import concourse.dve_ops as dops
    from concourse.dve_spec import Spec, Src0, C0, C1, C2, C3, sq, lower

    name = "EXP4A"
    if name in dops._SUB_OPCODE_FOR_NAME:
        return next(o for o in dops.OPS if o.name == name)
    p = ((C3 * Src0 + C2) * Src0 + C1) * Src0 + C0
    spec = Spec(body=dops._spill_c3_to_src1(sq(sq(p))))
    opcode = max(dops._SUB_OPCODE_FOR_NAME.values()) + 1
    assert opcode <= 31
    dops._SUB_OPCODE_FOR_NAME[name] = opcode
    shas = {}
    for ver in ("v3", "v4"):
        s = dops.DveOpSpec(
            name=name, opcode=opcode, uops=lower(spec, ver=ver), rd1_en=True
        )
        shas[ver] = s.sha(ver)
    op = dops.DveOp(name, spec, subdim=False, uops_sha=shas)
    dops.OPS.append(op)
    return op


EXP4 = _register_exp4()


class PatchedTileContext(TileContext):
    """This walrus build encodes at most 2 sync-waits per CTRL instruction,
    but the stock kernel-tail drain carries one wait per active proc.
    Distribute the waits across single-wait NOPs ahead of the drain."""

    def _drain_and_barrier(self, tick_clock, wait_clock):
        probe = self.nc.sync.nop(nofuse=True, hint="drain_waits").ins
        wait_clock.add_sem_waits(probe, ScopedClock({None: tick_clock.global_clock}))
        waits = list(probe.sync_info.on_wait) if probe.sync_info else []
        updates = list(probe.sync_info.on_update) if probe.sync_info else []
        probe.sync_info = mybir.SyncInfo(on_wait=waits[:1], on_update=updates)
        for k in range(1, len(waits)):
            nxt = self.nc.sync.nop(nofuse=True, hint=f"drain_waits_{k}").ins
            nxt.sync_info = mybir.SyncInfo(on_wait=[waits[k]], on_update=[])
        self.nc.sync.drain()
        self.nc.all_engine_barrier()
        popped = self.nc._tile_sem_poison_stack.pop()
        assert popped is self._sem_poison
        self.nc.clear_and_free_semaphores(list(self.sems.allocated().values()))
        self.nc.all_engine_barrier()


class EngineBalancer:
    """Greedy ns-load balancing between ACT and DVE for balancer-eligible ops."""

    def __init__(self, nc, c3_ap, dve_frac=0.5, dve_fixed_ns=70000.0):
        self.nc = nc
        self.c3 = c3_ap
        self.frac = dve_frac
        # pre-load DVE with its (un-balanceable) fixed work: bias copies,
        # mask muls, normalize muls, reciprocals
        self.load = {"ACT": 0.0, "DVE": dve_fixed_ns}

    def _pick(self, cost_act, cost_dve):
        if self.frac <= 0.0:
            self.load["ACT"] += cost_act
            return "ACT"
        # keep DVE/(DVE+ACT) ~ frac
        tot = self.load["ACT"] + self.load["DVE"] + 1.0
        if self.load["DVE"] / tot < self.frac:
            self.load["DVE"] += cost_dve
            return "DVE"
        self.load["ACT"] += cost_act
        return "ACT"

    def exp(self, out_ap, in_ap, width):
        eng = self._pick((width + 222) / 1.2, (width + 120) / 0.96)
        if eng == "ACT":
            self.nc.scalar.activation(out_ap, in_ap, AF.Exp, bias=0.0, scale=0.125)
        else:
            self.nc.vector._custom_dve(
                EXP4,
                out=out_ap,
                in0=in_ap,
                in1=self.c3,
                s0=EXP4_C[0],
                s1=EXP4_C[1],
                imm2=EXP4_C[2],
            )

    def copy(self, out_ap, in_ap, width):
        eng = self._pick((width + 222) / 1.2, (width + 120) / 0.96)
        if eng == "ACT":
            self.nc.scalar.copy(out=out_ap, in_=in_ap)
        else:
            self.nc.vector.tensor_copy(out=out_ap, in_=in_ap)


def build_program(
    nc,
    s_total=S,
    pt_bufs=4,
    score_bufs=2,
    feats_bufs=2,
    dve_frac=0.5,
    dve_fixed_ns=70000.0,
    time_reps=1,
):
    """Emit the per-core attention program. s_total must divide by 512."""
    nb = s_total // SQ  # number of 512-query blocks
    qT = nc.dram_tensor("qT", [D, s_total], BF16, kind="ExternalInput")
    kT = nc.dram_tensor("kT", [D, s_total], BF16, kind="ExternalInput")
    vT = nc.dram_tensor("vT", [D, s_total], BF16, kind="ExternalInput")
    wqT = nc.dram_tensor("wqT", [D, 256], BF16, kind="ExternalInput")
    wkT = nc.dram_tensor("wkT", [D, 256], BF16, kind="ExternalInput")
    wvT = nc.dram_tensor("wvT", [D, HB], BF16, kind="ExternalInput")
    woT = nc.dram_tensor("woT", [HB, D], BF16, kind="ExternalInput")
    bias_qk = nc.dram_tensor("bias_qk", [128, 2, 2], F32, kind="ExternalInput")
    bv_row = nc.dram_tensor("bv_row", [1, HB], BF16, kind="ExternalInput")
    ones128 = nc.dram_tensor("ones128", [1, 128], BF16, kind="ExternalInput")
    out = nc.dram_tensor("out", [s_total, D], F32, kind="ExternalOutput")

    with PatchedTileContext(nc) as tc:
        import contextlib

        with contextlib.ExitStack() as ctx:
            cpool = ctx.enter_context(tc.tile_pool(name="consts", bufs=1))
            stream = ctx.enter_context(tc.tile_pool(name="stream", bufs=3))
            qh_pool = ctx.enter_context(tc.tile_pool(name="qh", bufs=2))
            kv_pool = ctx.enter_context(tc.tile_pool(name="kv", bufs=1))
            pt_pool = ctx.enter_context(tc.tile_pool(name="pt", bufs=pt_bufs))
            sm_pool = ctx.enter_context(tc.tile_pool(name="sm", bufs=2))
            feats_pool = ctx.enter_context(tc.tile_pool(name="feats", bufs=feats_bufs))
            osb_pool = ctx.enter_context(tc.tile_pool(name="osb", bufs=2))
            # PSUM budget (8 banks): score slots 2 x [128,3,512] (6 banks)
            # + psf 2 x [65,512] (2 banks). Projections and Wo share the
            # score slots.
            ps_score = ctx.enter_context(
                tc.tile_pool(name="ps_score", bufs=score_bufs, space="PSUM")
            )
            ps_feat = ctx.enter_context(tc.tile_pool(name="ps_feat", bufs=2, space="PSUM"))

            # ---- constants / weights ----
            wq_sb = cpool.tile([128, 6, 256], BF16, tag="wq")
            wk_sb = cpool.tile([128, 6, 256], BF16, tag="wk")
            for dst, src in ((wq_sb, wqT), (wk_sb, wkT)):
                nc.sync.dma_start(
                    out=dst[:], in_=src[:].rearrange("(c p) m -> p c m", p=128)
                )
            wv_sb = cpool.tile([128, 6, HB], BF16, tag="wv")
            nc.sync.dma_start(
                out=wv_sb[:], in_=wvT[:].rearrange("(c p) m -> p c m", p=128)
            )
            wo_sb = cpool.tile([64, HPC, D], BF16, tag="wo")
            nc.sync.dma_start(out=wo_sb[:], in_=woT[:].rearrange("(h p) n -> p h n", p=64))
            bias_sb = cpool.tile([128, 2, 2], F32, tag="bias")
            nc.sync.dma_start(out=bias_sb[:], in_=bias_qk[:])
            bv_sb = cpool.tile([1, HB], BF16, tag="bv")
            nc.sync.dma_start(out=bv_sb[:], in_=bv_row[:])
            ones_row = cpool.tile([1, 128], BF16, tag="ones")
            nc.sync.dma_start(out=ones_row[:], in_=ones128[:])
            c3_sb = cpool.tile([128, 1], F32, tag="c3")
            nc.vector.memset(c3_sb[:], EXP4_C[3])
            # 0/1 causal masks for the 4 diagonal sk-tiles of a 512 block:
            # mask_m[p, f] = 1 where f >= p + 128*m else 0
            masks = cpool.tile([128, 4, SQ], BF16, tag="masks")
            nc.gpsimd.memset(masks[:], 0.0)
            for m in range(4):
                nc.gpsimd.affine_select(
                    out=masks[:, m, :],
                    in_=masks[:, m, :],
                    compare_op=mybir.AluOpType.is_gt,
                    fill=1.0,
                    base=128 * m,
                    pattern=[[-1, SQ]],
                    channel_multiplier=1,
                )

            bal = EngineBalancer(nc, c3_sb[:], dve_frac=dve_frac, dve_fixed_ns=dve_fixed_ns)

            for _rep in range(time_reps):
                kh01s = []  # [128, SQ] per block: heads 0,1 stacked
                kh2ds = []  # [128, SQ] per block: head 2 duplicated both halves
                vhs = []  # [128, 4, 3, 65] per block (Vh', ones col appended)

                for j in range(nb):
                    sq_lo = j * SQ
                    # ---- stream transposed activations for this block ----
                    qt = stream.tile([128, 6, SQ], BF16, tag="qt")
                    kt = stream.tile([128, 6, SQ], BF16, tag="kt")
                    vt = stream.tile([128, 6, SQ], BF16, tag="vt")
                    for dst, src in ((qt, qT), (kt, kT), (vt, vT)):
                        nc.sync.dma_start(
                            out=dst[:],
                            in_=src[:].rearrange("(c p) s -> p c s", p=128)[
                                :, :, sq_lo : sq_lo + SQ
                            ],
                        )

                    # ---- Q/K projections ----
                    qh01 = qh_pool.tile([128, SQ], BF16, tag="qh01")
                    qh2d = qh_pool.tile([128, SQ], BF16, tag="qh2d")
                    k01 = kv_pool.tile([128, SQ], BF16, tag=f"kh01_{j}")
                    k2d = kv_pool.tile([128, SQ], BF16, tag=f"kh2d_{j}")
                    kh01s.append(k01)
                    kh2ds.append(k2d)
                    for xt, wsb, o01, o2d, bi in (
                        (qt, wq_sb, qh01, qh2d, 0),
                        (kt, wk_sb, k01, k2d, 1),
                    ):
                        ps = ps_score.tile([128, 3, SQ], F32, tag="slot")
                        for half, mlo in ((0, 0), (1, 128)):
                            for c in range(6):
                                nc.tensor.matmul(
                                    ps[:, half, :],
                                    lhsT=wsb[:, c, mlo : mlo + 128],
                                    rhs=xt[:, c, :],
                                    start=(c == 0),
                                    stop=(c == 5),
                                )
                        nc.vector.tensor_scalar_add(o01[:], ps[:, 0, :], bias_sb[:, bi, 0:1])
                        nc.vector.tensor_scalar_add(o2d[:], ps[:, 1, :], bias_sb[:, bi, 1:2])

                    # ---- V projection -> natural [s, head, 64] + ones column ----
                    vj = kv_pool.tile([128, 4, HPC, HD + 1], BF16, tag=f"vh_{j}")
                    for sp in range(2):
                        psv2 = ps_score.tile([128, 3, SQ], F32, tag="slot")
                        for half in range(2):
                            st = 2 * sp + half
                            psv = psv2[:, half, 0:HB]
                            for c in range(6):
                                nc.tensor.matmul(
                                    psv,
                                    lhsT=vt[:, c, st * 128 : (st + 1) * 128],
                                    rhs=wv_sb[:, c, :],
                                    start=(c == 0),
                                    stop=False,
                                )
                            nc.tensor.matmul(
                                psv, lhsT=ones_row[:], rhs=bv_sb[:], start=False, stop=True
                            )
                            bal.copy(
                                vj[:, st, :, 0:HD],
                                psv.rearrange("p (h e) -> p h e", e=HD),
                                HB,
                            )
                            nc.vector.memset(vj[:, st, :, HD : HD + 1], 1.0)
                    vhs.append(vj)

                    # ---- attention: tile stream in 3-tile PSUM slots ----
                    # entry = (h, t, half, lo): h head, t sk-tile index,
                    # half row-group (0 -> rows 0-63, 1 -> rows 64-127),
                    # lo column restriction (diag tiles)
                    n_sk = 4 * (j + 1)
                    # entry = (h, t, half, lo, m): m = diag tile index or None
                    entries = []
                    for t in range(4 * j):  # off-diag h0,h1 interleaved
                        entries.append((0, t, 0, 0, None))
                        entries.append((1, t, 1, 0, None))
                    for m in range(4):  # diag h0,h1
                        entries.append((0, 4 * j + m, 0, 128 * m, m))
                        entries.append((1, 4 * j + m, 1, 128 * m, m))
                    h2half = 0
                    for t in range(4 * j):  # off-diag h2 (alternating halves)
                        entries.append((2, t, h2half, 0, None))
                        h2half ^= 1
                    for m in range(4):  # diag h2
                        entries.append((2, 4 * j + m, h2half, 128 * m, m))
                        h2half ^= 1

                    first_t = {0: 0, 1: 0, 2: 0}
                    last_seen = {h: max(i for i, e in enumerate(entries) if e[0] == h)
                                 for h in range(HPC)}
                    psf = {}
                    feats = feats_pool.tile([64, HPC, SQ], BF16, tag="feats")

                    def lhsT_k(h, t, half):
                        jj, tt = t // 4, t % 4
                        sl = slice(tt * 128, (tt + 1) * 128)
                        if h == 0:
                            return kh01s[jj][0:64, sl]
                        if h == 1:
                            return kh01s[jj][64:128, sl]
                        return kh2ds[jj][64 * half : 64 * (half + 1), sl]

                    def rhs_q(h, half, lo):
                        src = qh01 if h < 2 else qh2d
                        hh = h if h < 2 else half
                        return src[64 * hh : 64 * (hh + 1), lo:SQ]

                    def emit_pv(slot_entries, pt):
                        for i, (h, t, half, lo, m) in enumerate(slot_entries):
                            jj, tt = t // 4, t % 4
                            if h not in psf:
                                pf_new = ps_feat.tile([HD + 1, SQ], F32, tag="pf")
                                psf[h] = pf_new
                            nc.tensor.matmul(
                                psf[h][:, lo:SQ],
                                lhsT=vhs[jj][:, tt, h, :],
                                rhs=pt[:, i, lo:SQ],
                                start=(t == first_t[h]),
                                stop=(t == n_sk - 1),
                            )

                    prev = None  # (slot_entries, pt)
                    for s0 in range(0, len(entries), 3):
                        sl_entries = entries[s0 : s0 + 3]
                        pss = ps_score.tile([128, 3, SQ], F32, tag="slot")
                        pt = pt_pool.tile([128, 3, SQ], BF16, tag="pt")
                        for i, (h, t, half, lo, m) in enumerate(sl_entries):
                            nc.tensor.matmul(
                                pss[:, i, lo:SQ],
                                lhsT=lhsT_k(h, t, half),
                                rhs=rhs_q(h, half, lo),
                                start=True,
                                stop=True,
                            )
                        # exp: group contiguous full-width tiles; restricted
                        # (diag) tiles individually; mask every diag tile
                        i = 0
                        n = len(sl_entries)
                        while i < n:
                            lo = sl_entries[i][3]
                            if lo == 0:
                                i1 = i
                                while i1 + 1 < n and sl_entries[i1 + 1][3] == 0:
                                    i1 += 1
                                bal.exp(
                                    pt[:, i : i1 + 1, :], pss[:, i : i1 + 1, :],
                                    (i1 - i + 1) * SQ,
                                )
                                i = i1 + 1
                            else:
                                bal.exp(pt[:, i, lo:SQ], pss[:, i, lo:SQ], SQ - lo)
                                i += 1
                        for i, (h, t, half, lo, m) in enumerate(sl_entries):
                            if m is not None:
                                nc.vector.tensor_mul(
                                    pt[:, i, lo:SQ], pt[:, i, lo:SQ],
                                    masks[:, m, lo:SQ],
                                )
                        if prev is not None:
                            emit_pv(*prev)
                        prev = (sl_entries, pt)
                    if prev is not None:
                        emit_pv(*prev)

                    # ---- normalize each head ----
                    for h in range(HPC):
                        pf = psf[h]
                        recip = sm_pool.tile([1, SQ], F32, tag="recip")
                        nc.vector.reciprocal(recip[:], pf[HD : HD + 1, :])
                        rbc = sm_pool.tile([64, SQ], F32, tag="rbc")
                        nc.gpsimd.partition_broadcast(rbc[:], recip[:])
                        nc.vector.tensor_mul(feats[:, h, :], pf[0:HD, :], rbc[:])
                    psf.clear()

                    # ---- output projection (partial over local heads) ----
                    for st in range(4):
                        pso = ps_score.tile([128, 3, SQ], F32, tag="slot")
                        for h in range(HPC):
                            for i1, n0, nsz in ((0, 0, 512), (1, 512, 256)):
                                nc.tensor.matmul(
                                    pso[:, i1, 0:nsz],
                                    lhsT=feats[:, h, st * 128 : (st + 1) * 128],
                                    rhs=wo_sb[:, h, n0 : n0 + nsz],
                                    start=(h == 0),
                                    stop=(h == HPC - 1),
                                )
                        osb = osb_pool.tile([128, D], F32, tag="osb")
                        bal.copy(
                            osb[:], pso[:].rearrange("p a b -> p (a b)")[:, 0:D], D
                        )
                        nc.sync.dma_start(
                            out=out[sq_lo + st * 128 : sq_lo + (st + 1) * 128, :], in_=osb[:]
                        )

    return nc


def build_nc(s_total=S, **kw):
    from concourse import bacc

    nc = bacc.Bacc(num_devices=N_CORES)
    build_program(nc, s_total=s_total, **kw)
    nc.compile()
    return nc


# ---------------------------------------------------------------------------
# Host-side sharding / unsharding
# ---------------------------------------------------------------------------


def shard_inputs(q, k, v, Wq, bq, Wk, bk, Wv, bv, Wo, bo, s_total=S):
    """Build the 8 per-core input maps (numpy)."""
    in_maps = []
    qT = [np.ascontiguousarray(np.asarray(q)[b, :s_total].T) for b in range(B)]
    kTb = [np.ascontiguousarray(np.asarray(k)[b, :s_total].T) for b in range(B)]
    vTb = [np.ascontiguousarray(np.asarray(v)[b, :s_total].T) for b in range(B)]
    Wq, Wk, Wv, Wo = (np.asarray(x) for x in (Wq, Wk, Wv, Wo))
    bq, bk, bv = (np.asarray(x) for x in (bq, bk, bv))
    for c in range(N_CORES):
        b = c // CORES_PER_BATCH
        g = c % CORES_PER_BATCH
        lo, hi = HB * g, HB * (g + 1)
        # W[lo:lo+128] for heads 0,1; W[lo+128:lo+192] DUPLICATED for head 2
        wq_cols = np.concatenate(
            [Wq[lo : lo + 128].T, Wq[lo + 128 : hi].T, Wq[lo + 128 : hi].T], axis=1
        )
        wk_cols = np.concatenate(
            [Wk[lo : lo + 128].T, Wk[lo + 128 : hi].T, Wk[lo + 128 : hi].T], axis=1
        )
        bias_qk = np.zeros((128, 2, 2), np.float32)
        for i, bvec in enumerate((bq[lo:hi], bk[lo:hi])):
            bias_qk[:, i, 0] = bvec[:128]
            bias_qk[:64, i, 1] = bvec[128:]
            bias_qk[64:, i, 1] = bvec[128:]
        in_maps.append(
            {
                "qT": qT[b].astype(ml_dtypes.bfloat16),
                "kT": kTb[b].astype(ml_dtypes.bfloat16),
                "vT": vTb[b].astype(ml_dtypes.bfloat16),
                "wqT": np.ascontiguousarray(wq_cols).astype(ml_dtypes.bfloat16),
                "wkT": np.ascontiguousarray(wk_cols).astype(ml_dtypes.bfloat16),
                "wvT": np.ascontiguousarray(Wv[lo:hi].T).astype(ml_dtypes.bfloat16),
                "woT": np.ascontiguousarray(Wo[:, lo:hi].T).astype(ml_dtypes.bfloat16),
                "bias_qk": bias_qk,
                "bv_row": np.ascontiguousarray(bv[lo:hi])[None, :].astype(ml_dtypes.bfloat16),
                "ones128": np.ones((1, 128), ml_dtypes.bfloat16),
            }
        )
    return in_maps


def unshard_outputs(results, bo, s_total=S):
    """Sum the 4 row-parallel partials per batch and add bo."""
    bo = np.asarray(bo, np.float32)
    out = np.empty((B, s_total, D), np.float32)
    for b in range(B):
        acc = results[b * CORES_PER_BATCH]["out"].astype(np.float32)
        for c in range(b * CORES_PER_BATCH + 1, (b + 1) * CORES_PER_BATCH):
            acc = acc + results[c]["out"]
        out[b] = acc + bo
    return out


def kernel(q, k, v, mask, Wq, bq, Wk, bk, Wv, bv, Wo, bo):
    """Full-input entry point: returns [B, S, D] float32."""
    from concourse.bass_utils import run_bass_kernel_spmd

    nc = build_nc()
    in_maps = shard_inputs(q, k, v, Wq, bq, Wk, bk, Wv, bv, Wo, bo)
    res = run_bass_kernel_spmd(nc, in_maps, list(range(N_CORES)))
    return unshard_outputs(res.results, bo)
